# revision 42
# baseline (speedup 1.0000x reference)
"""Trainium2 Bass kernel for nn_MemoryRetriever (cross-attention memory retriever).

Strategy (v3):
- Host-side mask compaction: masked-out keys (~50%) are dropped on the host;
  survivors are dealt evenly to the 8 cores (zero-padded to SKC keys/core,
  pads confined to each core's last 512-key chunk and killed by a -30 exp
  bias).
- Precision plan (rel-err budget ~1.6e-2): everything is bf16 except three
  fp8 uses that attention averaging washes out: the mem operand of the K/V
  projections, the V projection weights (two-term hi+lo fp8, lo applied to
  mem/16 so both DoubleRow matmuls accumulate in one PSUM group), and the
  exp outputs pt (which feed fp8 DoubleRow numerator and denominator
  reductions). K projection weights use the same two-term fp8 split.
  Q path, scores, output path are bf16; the collective payload is bf16.
- PE: K/V projections run at 2x bf16 speed via paired-fp8 DoubleRow; the
  denominator is a DoubleRow matmul against a head-selector so no vector
  accumulation is needed; numerator is DoubleRow over key-tile pairs.
- Schedule: per-chunk software pipeline -- rope(o) unlocks head h=o-1
  scores+exp immediately; next chunk's projection units and lagged
  denominator matmuls fill all gaps. GPSIMD only touches SBUF (squares,
  broadcasts); PSUM reads happen on PE/Act/DVE only.
"""

import os
import sys
import numpy as np

sys.path.insert(0, "/opt/trn_rl_repo")

DIM = 1024
HEADS = 8
HD = 128
SQ = 512
N_CORES = 8
QS = SQ // N_CORES
EPS = 1e-6
SCALE = 1.0 / np.sqrt(128.0)
WS = 16.0            # host-side weight scale (fp8 subnormal avoidance)
SHIFT = -3.0         # exp(score + SHIFT): keeps fp8e4 pt in range
PADB = -30.0         # exp bias for padded keys
CHT = 4              # key tiles (128) per chunk

_cache = {}


def _build(skc=2048):
    key = ("nc", skc)
    if key in _cache:
        return _cache[key]

    import concourse.bass as bass
    import concourse.tile as tile
    from concourse import mybir, bacc

    f32 = mybir.dt.float32
    bf16 = mybir.dt.bfloat16
    fp8 = mybir.dt.float8e4
    AF = mybir.ActivationFunctionType
    DR = mybir.MatmulPerfMode.DoubleRow

    TT = skc // 128          # key tiles per core
    NCH = TT // CHT          # chunks per core
    assert skc % (CHT * 128) == 0

    _sim = os.environ.get("KSIM", "0") == "1"

    nc = bacc.Bacc("TRN2", target_bir_lowering=False, debug=False,
                   num_devices=N_CORES)

    def din(name, shape, dt=f32):
        return nc.dram_tensor(name, list(shape), dt, kind="ExternalInput").ap()

    # per-core sharded inputs
    memT = din("memT", [DIM, skc], fp8)       # compacted mem shard
    memL = din("memL", [DIM, skc], fp8)       # mem/16 (two-term lo operand)
    cstk = din("cstk", [HD, 2, skc], bf16)    # K rope cos/sin (pair-dup rows)
    mbt = din("mbt", [128, TT])               # exp bias per (key%128, tile)
    # shared inputs
    xt = din("xt", [128, 8, SQ], bf16)        # x.T tiled [p,i,q]
    wq = din("wq", [128, 8, 8, 128], bf16)    # [p,i,o,m] = WS*Wq.T[i*128+p, .]
    wk = din("wk", [128, 8, 8, 128], fp8)     # two-term hi
    wk2 = din("wk2", [128, 8, 8, 128], fp8)   # two-term lo (16x residual)
    wv = din("wv", [128, 8, DIM], fp8)        # [p,i,d] hi
    wv2 = din("wv2", [128, 8, DIM], fp8)      # lo
    wo = din("wo", [128, 8, 8, 128], bf16)
    ctq = din("ctq", [128, 8, SQ], bf16)      # q rope cos (gq*gk folded)
    stq = din("stq", [128, 8, SQ], bf16)
    bo_t = din("bo_t", [128, 8])              # bo + Wo@bv folded
    pmat = din("pmat", [128, 128], bf16)      # P.T for rope pair swap (+-1)
    ones_c = din("ones_c", [128, 1], bf16)
    sel = din("sel", [128, 2, 64], fp8)       # den head selector

    outT = nc.dram_tensor("outT", [DIM, SQ], f32, kind="ExternalOutput").ap()
    cat = nc.dram_tensor("cat", [DIM + HEADS, SQ], f32)
    cat_sh = nc.dram_tensor("cat_sh", [DIM + HEADS, SQ], f32,
                            addr_space="Shared")

    MUL = mybir.AluOpType.mult
    ADD = mybir.AluOpType.add
    POW = mybir.AluOpType.pow

    with tile.TileContext(nc) as tc:
        ctx_pools = []   # list of (cm, entered)

        def pool(name, bufs, space=None):
            kw = dict(name=name, bufs=bufs)
            if space:
                kw["space"] = space
            cm = tc.tile_pool(**kw)
            entered = cm.__enter__()
            ctx_pools.append((cm, entered))
            return entered

        def close_pool(entered):
            for i, (cm, e) in enumerate(ctx_pools):
                if e is entered:
                    cm.__exit__(None, None, None)
                    ctx_pools.pop(i)
                    return
            raise KeyError("pool not found")

        consts = pool("consts", 1)
        resid = pool("resid", 1)
        pp_den = pool("pp_den", 1, space="PSUM")  # den [8,512]
        pp_s = pool("pp_s", 2, space="PSUM")      # scores [128,2,512]
        wpool = pool("wpool", 2)    # small working tiles (rope/squares)
        spool = pool("spool", 1)    # [1,n] rs scalars
        pp_all = pool("pp_all", 2, space="PSUM")  # [128,512] proj/V/swap psum
        pp_sq2 = pool("pp_sq2", 1, space="PSUM")  # sumsq [1,512]
        kpool = pool("kpool", 2)

        _cnt = [0]

        def cload(shape, dt, src, via=nc.sync, into=None):
            _cnt[0] += 1
            t = (into or consts).tile(shape, dt, tag=f"c{_cnt[0]}")
            via.dma_start(t[:], src)
            return t

        qT = resid.tile([128, 8, SQ], bf16)         # rope'd normalized Q
        kra = resid.tile([128, 8, CHT, 128], bf16)  # rope'd normalized K (ping)
        krb = resid.tile([128, 8, CHT, 128], bf16)  # (pong)
        v_sb = resid.tile([128, TT, DIM], fp8)
        nacc = resid.tile([128, 8, SQ], f32)
        dacc = resid.tile([8, SQ], f32)

        den_ps = pp_den.tile([8, SQ], f32)

        def rs_broadcast(ps_sq, n):
            """rs = (sumsq_raw/DIM + eps*WS^2)^-0.5 (DVE pow) then bcast.
            y is kept raw (x WS); dividing by the raw rms normalizes WS away
            up to the folded eps."""
            t = spool.tile([1, n], f32, tag="lnm")
            nc.vector.tensor_scalar(t[:], ps_sq[:], 1.0 / DIM,
                                    EPS * WS * WS, MUL, ADD)
            rs = spool.tile([1, n], bf16, tag="rs")
            nc.vector.tensor_scalar(rs[:], t[:], -0.5, 1.0, POW, MUL)
            rsb = wpool.tile([128, n], bf16, tag="rsb")
            nc.gpsimd.partition_broadcast(rsb[:], rs[:])
            return rsb

        def rope_thunks(ysrc, n, rsb_of, ct_of, st_of, out_of,
                        fold_tables=False):
            """per-o rope emission thunks (thunk 0 computes folded tables)."""
            box = {}

            def ro(o):
                if fold_tables:
                    if o == 0:
                        ct_r = wpool.tile([128, n], bf16, tag="ctr")
                        nc.vector.tensor_mul(ct_r[:], ct_of(0), rsb_of())
                        st_r = wpool.tile([128, n], bf16, tag="str")
                        nc.vector.tensor_mul(st_r[:], st_of(0), rsb_of())
                        box["ct"], box["st"] = ct_r, st_r
                    ykn, ct_o, st_o = ysrc[:, o, :], box["ct"][:], box["st"][:]
                else:
                    ykn = wpool.tile([128, n], bf16, tag="ykn")
                    nc.vector.tensor_mul(ykn[:], ysrc[:, o, :], rsb_of())
                    ct_o, st_o = ct_of(o), st_of(o)
                ys = wpool.tile([128, n], bf16, tag="ys")
                nc.vector.tensor_mul(ys[:], ykn, st_o)
                swp = pp_all.tile([128, n], f32, tag="pp")
                nc.tensor.matmul(swp[:], pt_s[:], ys[:])
                yc = wpool.tile([128, n], bf16, tag="yc")
                nc.vector.tensor_mul(yc[:], ykn, ct_o)
                nc.vector.tensor_add(out_of(o), yc[:], swp[:])

            return [lambda o=o: ro(o) for o in range(8)]

        def weave(*lanes):
            """emit lanes with proportional progress (round-robin)."""
            lanes = [list(ln) for ln in lanes if ln]
            total = sum(len(ln) for ln in lanes)
            idx = [0] * len(lanes)
            for step in range(1, total + 1):
                for li, ln in enumerate(lanes):
                    want = (step * len(ln) + total - 1) // total
                    while idx[li] < min(want, len(ln)):
                        ln[idx[li]]()
                        idx[li] += 1

        def sumsq_unit(ydst, o, ps_sq):
            ysq = wpool.tile([128, 512], bf16, tag="ysq")
            nc.gpsimd.tensor_mul(ysq[:], ydst[:, o, :], ydst[:, o, :])
            nc.tensor.matmul(ps_sq[:], ones_s[:], ysq[:],
                             start=(o == 0), stop=(o == 7))

        def unit_K2(hi, lo, mh, ml, ydst, o, ps_sq):
            """two-term fp8 DoubleRow projection block + copy + sumsq."""
            ps = pp_all.tile([128, 512], f32, tag="pp")
            for pr in range(4):
                nc.tensor.matmul(ps[:], hi[:, 2 * pr:2 * pr + 2, o, :],
                                 mh[:, 2 * pr:2 * pr + 2, :],
                                 start=(pr == 0), stop=False, perf_mode=DR)
            for pr in range(4):
                nc.tensor.matmul(ps[:], lo[:, 2 * pr:2 * pr + 2, o, :],
                                 ml[:, 2 * pr:2 * pr + 2, :],
                                 start=False, stop=(pr == 3), perf_mode=DR)
            nc.scalar.activation(ydst[:, o, :], ps[:], AF.Copy)
            sumsq_unit(ydst, o, ps_sq)

        def unit_Kbf(w_s, src, ydst, o, ps_sq):
            """plain bf16 projection block (Q path)."""
            ps = pp_all.tile([128, 512], f32, tag="pp")
            for i in range(8):
                nc.tensor.matmul(ps[:], w_s[:, i, o, :], src[:, i, :],
                                 start=(i == 0), stop=(i == 7))
            nc.scalar.activation(ydst[:, o, :], ps[:], AF.Copy)
            sumsq_unit(ydst, o, ps_sq)

        def unit_V(mh, ml, gt, t):
            """two-term fp8 DoubleRow V projection for key tile t."""
            for oh in range(2):
                ps = pp_all.tile([128, 512], f32, tag="pp")
                for pr in range(4):
                    nc.tensor.matmul(
                        ps[:], mh[:, 2 * pr:2 * pr + 2, t * 128:(t + 1) * 128],
                        wv_s[:, 2 * pr:2 * pr + 2, oh * 512:(oh + 1) * 512],
                        start=(pr == 0), stop=False, perf_mode=DR)
                for pr in range(4):
                    nc.tensor.matmul(
                        ps[:], ml[:, 2 * pr:2 * pr + 2, t * 128:(t + 1) * 128],
                        wl_s[:, 2 * pr:2 * pr + 2, oh * 512:(oh + 1) * 512],
                        start=False, stop=(pr == 3), perf_mode=DR)
                nc.vector.tensor_copy(
                    v_sb[:, gt, oh * 512:(oh + 1) * 512], ps[:])

        def group_SE(c, kr, h, p2):
            """scores + exp for (head h, tile pair p2) of chunk c."""
            ps_s = pp_s.tile([128, 2, 512], f32, tag="ps_s")
            for tt in range(2):
                nc.tensor.matmul(ps_s[:, tt, :], kr[:, h, p2 * 2 + tt, :],
                                 qT[:, h, :])
            g0 = c * CHT + p2 * 2
            if c == NCH - 1:
                # pads live here: per-tile exp bias
                for tt in range(2):
                    nc.scalar.activation(pt_all[:, h, g0 + tt, :],
                                         ps_s[:, tt, :], AF.Exp, scale=SCALE,
                                         bias=mb_s[:, g0 + tt:g0 + tt + 1])
            else:
                nc.scalar.activation(pt_all[:, h, g0:g0 + 2, :], ps_s[:],
                                     AF.Exp, scale=SCALE,
                                     bias=mb_s[:, g0:g0 + 1])

        def den_mm(c, h, p2):
            gp = c * 2 + p2
            nc.tensor.matmul(den_ps[:], sel_s[:, :, h * 8:h * 8 + 8],
                             pt_all[:, h, gp * 2:gp * 2 + 2, :], perf_mode=DR,
                             start=(c == 0 and p2 == 0 and h == 0),
                             stop=(c == NCH - 1 and p2 == 1 and h == 7))

        def numer(h, pp):
            ps_n = pp.tile([128, SQ], f32, tag="pp")
            for p in range(TT // 2):
                nc.tensor.matmul(ps_n[:],
                                 v_sb[:, 2 * p:2 * p + 2, h * 128:(h + 1) * 128],
                                 pt_all[:, h, 2 * p:2 * p + 2, :],
                                 start=(p == 0), stop=(p == TT // 2 - 1),
                                 perf_mode=DR)
            nc.vector.tensor_copy(nacc[:, h, :], ps_n[:])
            nc.gpsimd.dma_start(
                cat[h * 128:(h + 1) * 128, :].rearrange(
                    "(a p) q -> p a q", p=128),
                nacc[:, h:h + 1, :])

        # ---- loads: SP queue carries the Q/K critical path ----
        wq_s = None  # placed in qpool below
        qpool = pool("qpool", 1)
        wq_s = cload([128, 8, 8, 128], bf16, wq, into=qpool)
        xt_s = qpool.tile([128, 8, SQ], bf16, tag="xt")
        nc.sync.dma_start(xt_s[:], xt)
        wk_s = cload([128, 8, 8, 128], fp8, wk)
        wl2_s = cload([128, 8, 8, 128], fp8, wk2)
        ctq_s = cload([128, 8, SQ], bf16, ctq, via=nc.gpsimd, into=qpool)
        stq_s = cload([128, 8, SQ], bf16, stq, via=nc.gpsimd, into=qpool)
        wv_s = cload([128, 8, DIM], fp8, wv, via=nc.gpsimd)
        wl_s = cload([128, 8, DIM], fp8, wv2, via=nc.gpsimd)
        pt_s = cload([128, 128], bf16, pmat, via=nc.gpsimd)
        ones_s = cload([128, 1], bf16, ones_c, via=nc.gpsimd)
        sel_s = cload([128, 2, 64], fp8, sel, via=nc.gpsimd)
        mb_s = cload([128, TT], f32, mbt, via=nc.gpsimd)
        bo_s = cload([128, 8], f32, bo_t, via=nc.gpsimd)

        # =========== pipelined chunk loop (Q phase = prologue) ===========
        cw = CHT * 128
        st = {}

        def s1_load(c):
            c0 = c * cw
            memt = kpool.tile([128, 8, cw], fp8, tag="memt")
            nc.sync.dma_start(
                memt[:], memT[:, c0:c0 + cw].rearrange("(i p) t -> p i t", p=128))
            meml = kpool.tile([128, 8, cw], fp8, tag="meml")
            nc.sync.dma_start(
                meml[:], memL[:, c0:c0 + cw].rearrange("(i p) t -> p i t", p=128))
            cs_t = kpool.tile([128, 2, cw], bf16, tag="cstk")
            nc.sync.dma_start(cs_t[:], cstk[:, :, c0:c0 + cw])
            kr = kra if c % 2 == 0 else krb
            return dict(memt=memt, meml=meml, ctk=cs_t[:, 0, :],
                        stk=cs_t[:, 1, :], kr=kr)

        def chunk_units(c):
            st[c] = s1_load(c)
            ps_sq = pp_sq2.tile([1, cw], f32, tag="pssq")
            st[c]["ps_sq"] = ps_sq
            ykt = kpool.tile([128, 8, 512], bf16, tag="yk")
            st[c]["yk"] = ykt
            units = []
            for o in range(8):
                units.append(lambda o=o, c=c: unit_K2(
                    wk_s, wl2_s, st[c]["memt"], st[c]["meml"],
                    st[c]["yk"], o, st[c]["ps_sq"]))
                if o % 2 == 1:
                    units.append(lambda o=o, c=c: unit_V(
                        st[c]["memt"], st[c]["meml"],
                        c * CHT + o // 2, o // 2))
            return units

        def chunk_rope_thunks(c):
            kr = st[c]["kr"]
            return rope_thunks(
                st[c]["yk"], cw, lambda c=c: st[c]["rsb"][:],
                lambda o, c=c: st[c]["ctk"][:],
                lambda o, c=c: st[c]["stk"][:],
                lambda o, kr=kr: kr[:, o, :, :], fold_tables=True)

        # Q prologue: Q proj, then Q rope woven with chunk-0 proj
        ps_sqq = pp_sq2.tile([1, SQ], f32, tag="pssq")
        yq = kpool.tile([128, 8, SQ], bf16, tag="yk")
        units0 = chunk_units(0)
        for o in range(8):
            unit_Kbf(wq_s, xt_s, yq, o, ps_sqq)
        rsb_q = rs_broadcast(ps_sqq, SQ)
        qrope = rope_thunks(yq, SQ, lambda: rsb_q[:],
                            lambda o: ctq_s[:, o, :], lambda o: stq_s[:, o, :],
                            lambda o: qT[:, o, :])
        weave(units0, qrope)
        close_pool(qpool)
        ptpool = pool("ptpool", 1)
        pt_all = ptpool.tile([128, 8, TT, SQ], fp8)  # exp(scores+shift)
        st[0]["rsb"] = rs_broadcast(st[0]["ps_sq"], cw)

        # pipelined chunk stream: rope(c,o) -> scores/exp(c,h=o-1) -> dens
        # (lagged) with next chunk's proj units spread throughout
        for c in range(NCH):
            last = c == NCH - 1
            ropes = chunk_rope_thunks(c)
            units = chunk_units(c + 1) if not last else []
            kr = kra if c % 2 == 0 else krb
            denq = []          # lagged den emission queue
            ui = 0

            def unit_step(frac, n_slots=12):
                nonlocal ui
                want = min(len(units), (frac * len(units)) // n_slots + 1)
                while ui < want:
                    units[ui]()
                    ui += 1

            slot = 0
            for o in range(8):
                unit_step(slot)
                ropes[o]()
                slot += 1
                for h in ([o - 1] if o >= 1 else []):
                    for p2 in range(2):
                        group_SE(c, kr, h, p2)
                        denq.append((c, h, p2))
                        while len(denq) > 2:
                            den_mm(*denq.pop(0))
                    if last:
                        numer(h, pp_all)
                    unit_step(slot)
                    slot += 1
            for h in (7,):
                for p2 in range(2):
                    group_SE(c, kr, h, p2)
                    denq.append((c, h, p2))
                    while len(denq) > 2:
                        den_mm(*denq.pop(0))
                if last:
                    numer(h, pp_all)
                unit_step(slot)
                slot += 1
            while ui < len(units):
                units[ui]()
                ui += 1
            while denq:
                den_mm(*denq.pop(0))
            if not last:
                st[c + 1]["rsb"] = rs_broadcast(st[c + 1]["ps_sq"], cw)

        nc.scalar.activation(dacc[:], den_ps[:], AF.Copy)
        nc.gpsimd.dma_start(cat[DIM:DIM + HEADS, :], dacc[:])

        # =========== reduce across cores ===========
        if _sim:
            nc.gpsimd.dma_start(cat_sh[0:512, :], cat[0:512, :])
            nc.gpsimd.dma_start(cat_sh[512:DIM + HEADS, :],
                                cat[512:DIM + HEADS, :])
        else:
            nc.gpsimd.collective_compute(
                "AllReduce", mybir.AluOpType.add,
                replica_groups=[list(range(N_CORES))],
                ins=[cat[:]], outs=[cat_sh[:]])

        for p in (kpool, pp_sq2, pp_all, spool, wpool, pp_s):
            close_pool(p)

        # =========== per-core output projection on its query slice ==========
        tail = pool("tail", 1)
        pp_t = pool("pp_t", 2, space="PSUM")
        wo_s = cload([128, 8, 8, 128], bf16, wo, into=tail)
        nred = tail.tile([128, 8, QS], f32)
        dred = tail.tile([1, HEADS, QS], f32)
        pid = nc.sync.partition_id()
        qoff = pid * QS
        nc.sync.dma_start(
            nred[:],
            cat_sh[0:DIM, bass.ds(qoff, QS)].rearrange("(h p) q -> p h q", p=128))
        nc.sync.dma_start(dred[:], cat_sh[DIM:DIM + HEADS, bass.ds(qoff, QS)])
        rd = tail.tile([1, HEADS, QS], f32)
        nc.vector.reciprocal(rd[:], dred[:])
        rdb = tail.tile([128, HEADS, QS], f32)
        nc.gpsimd.partition_broadcast(rdb[:], rd[:])
        nsc = tail.tile([128, 8, QS], bf16)
        nc.vector.tensor_mul(nsc[:], nred[:], rdb[:])
        out_sb = tail.tile([128, 8, QS], f32)
        for e in range(8):
            ps_o = pp_t.tile([128, QS], f32, tag="ppo")
            for o in range(8):
                nc.tensor.matmul(ps_o[:], wo_s[:, o, e, :], nsc[:, o, :],
                                 start=(o == 0), stop=(o == 7))
            nc.scalar.activation(out_sb[:, e, :], ps_o[:], AF.Identity,
                                 scale=1.0 / (WS * WS), bias=bo_s[:, e:e + 1])
        nc.sync.dma_start(
            outT.rearrange("(e p) q -> p e q", p=128)[:, :, 0:QS], out_sb[:])

        for cm, _ in reversed(ctx_pools):
            cm.__exit__(None, None, None)

    nc.compile()
    _cache[key] = nc
    _cache["nc"] = nc
    return nc


def _skc_for(nkeep):
    return max(CHT * 128, int(np.ceil(nkeep / (N_CORES * 512))) * 512)


def _prep(x, mem, mask, cos_q, sin_q, cos_k, sin_k,
          Wq, bq, Wk, bk, Wv, bv, Wo, bo, gq, gk):
    import ml_dtypes
    f = np.float32
    bf = ml_dtypes.bfloat16
    f8 = ml_dtypes.float8_e4m3
    x = np.asarray(x, f).reshape(SQ, DIM)
    mem = np.asarray(mem, f).reshape(-1, DIM)
    mask = np.asarray(mask).reshape(-1)
    cos_q = np.asarray(cos_q, f)
    sin_q = np.asarray(sin_q, f)
    cos_k = np.asarray(cos_k, f)
    sin_k = np.asarray(sin_k, f)
    Wq, Wk, Wv, Wo = (np.asarray(w, f) for w in (Wq, Wk, Wv, Wo))
    bq, bk, bv, bo, gq, gk = (np.asarray(v, f) for v in (bq, bk, bv, bo, gq, gk))

    if not np.allclose(gk, 1.0):
        gkp = gk.reshape(-1, 2)
        assert np.allclose(gkp[:, 0], gkp[:, 1]), "unsupported non-pairwise gk"
    assert np.allclose(bq, 0) and np.allclose(bk, 0), \
        "kernel specialized for zero q/k biases"

    idx = np.flatnonzero(mask)
    nkeep = len(idx)
    skc = _skc_for(nkeep)
    TT = skc // 128
    base, rem = divmod(nkeep, N_CORES)
    counts = [base + (1 if c < rem else 0) for c in range(N_CORES)]
    offs = np.concatenate([[0], np.cumsum(counts)])

    def tile_w(WT, dt):  # [1024,1024] (in,out of W.T) -> [p, i, o, m], scaled
        return np.ascontiguousarray(
            WT.reshape(8, 128, 8, 128).transpose(1, 0, 2, 3)).astype(dt)

    def two_term(WT):
        hi = (WT).astype(f8)
        lo = ((WT - hi.astype(f)) * 16.0).astype(f8)
        return hi.astype(f), lo.astype(f)

    ii = np.arange(128)
    jj = ii // 2
    partner = ii ^ 1

    # fold gq (and pairwise gk) into the q rope tables; sin pairs with
    # partner's gq
    gq_t = (gq * gk).reshape(8, 128)
    gq_sin = (gq.reshape(8, 128)[:, partner] * gk.reshape(8, 128))
    cq = cos_q[:, jj].T                # [128, SQ]
    sq = sin_q[:, jj].T
    ctq = np.ascontiguousarray(
        (cq[None, :, :] * gq_t[:, :, None]).transpose(1, 0, 2)).astype(bf)
    stq = np.ascontiguousarray(
        (sq[None, :, :] * gq_sin[:, :, None]).transpose(1, 0, 2)).astype(bf)

    PT = np.zeros((128, 128), f)
    even = ii[ii % 2 == 0]
    PT[even + 1, even] = -1.0
    PT[even, even + 1] = 1.0

    selm = np.zeros((128, 2, 64), f)
    for h in range(8):
        selm[:, :, h * 8 + h] = 1.0

    bo_f = bo + Wo @ bv

    wkh, wkl = two_term(Wk.T * WS)
    wvh, wvl = two_term(Wv.T * WS)

    shared = {
        "xt": np.ascontiguousarray(
            x.T.reshape(8, 128, SQ).transpose(1, 0, 2)).astype(bf),
        "wq": tile_w(Wq.T * WS, bf),
        "wk": tile_w(wkh, f8), "wk2": tile_w(wkl, f8),
        "wo": tile_w(Wo.T * WS, bf),
        "wv": np.ascontiguousarray(
            wvh.reshape(8, 128, DIM).transpose(1, 0, 2)).astype(f8),
        "wv2": np.ascontiguousarray(
            wvl.reshape(8, 128, DIM).transpose(1, 0, 2)).astype(f8),
        "ctq": ctq, "stq": stq,
        "bo_t": np.ascontiguousarray(bo_f.reshape(8, 128).T),
        "pmat": PT.astype(bf),
        "ones_c": np.ones((128, 1), bf),
        "sel": selm.astype(f8),
    }

    ckT = cos_k[:, jj].T.astype(f)     # [128, SK]
    skT = sin_k[:, jj].T.astype(f)

    in_maps = []
    for c in range(N_CORES):
        keys = idx[offs[c]:offs[c + 1]]
        n = len(keys)
        m = dict(shared)
        memc = np.zeros((DIM, skc), f8)
        memc[:, :n] = mem[keys].T.astype(f8)
        meml = np.zeros((DIM, skc), f8)
        meml[:, :n] = (mem[keys].T / 16.0).astype(f8)
        cstk = np.zeros((HD, 2, skc), bf)
        cstk[:, 0, :n] = ckT[:, keys].astype(bf)
        cstk[:, 1, :n] = skT[:, keys].astype(bf)
        mb = np.full((skc,), PADB, f)
        mb[:n] = SHIFT
        m["memT"] = memc
        m["memL"] = meml
        m["cstk"] = cstk
        m["mbt"] = np.ascontiguousarray(mb.reshape(TT, 128).T)
        in_maps.append(m)
    return in_maps


def _assemble(outTs):
    parts = [np.asarray(outTs[c])[:, 0:QS].T for c in range(N_CORES)]
    out = np.concatenate(parts, axis=0)
    return out[None].astype(np.float32)


def kernel(**inputs):
    from concourse.bass_utils import run_bass_kernel_spmd
    in_maps = _prep(**inputs)
    skc = in_maps[0]["memT"].shape[1]
    nc = _build(skc)
    res = run_bass_kernel_spmd(nc, in_maps, list(range(N_CORES)))
    return _assemble([res.results[c]["outT"] for c in range(N_CORES)])


# revision 43
# speedup vs baseline: 1.0411x; 1.0411x over previous
"""Trainium2 Bass kernel for nn_MemoryRetriever (cross-attention memory retriever).

Strategy (v3):
- Host-side mask compaction: masked-out keys (~50%) are dropped on the host;
  survivors are dealt evenly to the 8 cores (zero-padded to SKC keys/core,
  pads confined to each core's last 512-key chunk and killed by a -30 exp
  bias).
- Precision plan (rel-err budget ~1.6e-2): everything is bf16 except three
  fp8 uses that attention averaging washes out: the mem operand of the K/V
  projections, the V projection weights (two-term hi+lo fp8, lo applied to
  mem/16 so both DoubleRow matmuls accumulate in one PSUM group), and the
  exp outputs pt (which feed fp8 DoubleRow numerator and denominator
  reductions). K projection weights use the same two-term fp8 split.
  Q path, scores, output path are bf16; the collective payload is bf16.
- PE: K/V projections run at 2x bf16 speed via paired-fp8 DoubleRow; the
  denominator is a DoubleRow matmul against a head-selector so no vector
  accumulation is needed; numerator is DoubleRow over key-tile pairs.
- Schedule: per-chunk software pipeline -- rope(o) unlocks head h=o-1
  scores+exp immediately; next chunk's projection units and lagged
  denominator matmuls fill all gaps. GPSIMD only touches SBUF (squares,
  broadcasts); PSUM reads happen on PE/Act/DVE only.
"""

import os
import sys
import numpy as np

sys.path.insert(0, "/opt/trn_rl_repo")

DIM = 1024
HEADS = 8
HD = 128
SQ = 512
N_CORES = 8
QS = SQ // N_CORES
EPS = 1e-6
SCALE = 1.0 / np.sqrt(128.0)
WS = 16.0            # host-side weight scale (fp8 subnormal avoidance)
SHIFT = -3.0         # exp(score + SHIFT): keeps fp8e4 pt in range
PADB = -30.0         # exp bias for padded keys
CHT = 4              # key tiles (128) per chunk

_cache = {}


def _build(skc=2048):
    key = ("nc", skc)
    if key in _cache:
        return _cache[key]

    import concourse.bass as bass
    import concourse.tile as tile
    from concourse import mybir, bacc

    f32 = mybir.dt.float32
    bf16 = mybir.dt.bfloat16
    fp8 = mybir.dt.float8e4
    AF = mybir.ActivationFunctionType
    DR = mybir.MatmulPerfMode.DoubleRow

    TT = skc // 128          # key tiles per core
    NCH = TT // CHT          # chunks per core
    assert skc % (CHT * 128) == 0

    _sim = os.environ.get("KSIM", "0") == "1"

    nc = bacc.Bacc("TRN2", target_bir_lowering=False, debug=False,
                   num_devices=N_CORES)

    def din(name, shape, dt=f32):
        return nc.dram_tensor(name, list(shape), dt, kind="ExternalInput").ap()

    # per-core sharded inputs
    memT = din("memT", [DIM, skc], fp8)       # compacted mem shard
    memL = din("memL", [DIM, skc], fp8)       # mem/16 (two-term lo operand)
    cstk = din("cstk", [HD, 2, skc], bf16)    # K rope cos/sin (pair-dup rows)
    mbt = din("mbt", [128, TT])               # exp bias per (key%128, tile)
    # shared inputs
    xt = din("xt", [128, 8, SQ], bf16)        # x.T tiled [p,i,q]
    wq = din("wq", [128, 8, 8, 128], bf16)    # [p,i,o,m] = WS*Wq.T[i*128+p, .]
    wk = din("wk", [128, 8, 8, 128], fp8)     # two-term hi
    wk2 = din("wk2", [128, 8, 8, 128], fp8)   # two-term lo (16x residual)
    wv = din("wv", [128, 8, DIM], fp8)        # [p,i,d] hi
    wv2 = din("wv2", [128, 8, DIM], fp8)      # lo
    wo = din("wo", [128, 8, 8, 128], bf16)
    ctq = din("ctq", [128, 8, SQ], bf16)      # q rope cos (gq*gk folded)
    stq = din("stq", [128, 8, SQ], bf16)
    bo_t = din("bo_t", [128, 8])              # bo + Wo@bv folded
    pmat = din("pmat", [128, 128], bf16)      # P.T for rope pair swap (+-1)
    ones_c = din("ones_c", [128, 1], bf16)
    sel = din("sel", [128, 2, 64], fp8)       # den head selector

    outT = nc.dram_tensor("outT", [DIM, SQ], f32, kind="ExternalOutput").ap()
    cat = nc.dram_tensor("cat", [DIM + HEADS, SQ], f32)
    cat_sh = nc.dram_tensor("cat_sh", [DIM + HEADS, SQ], f32,
                            addr_space="Shared")

    MUL = mybir.AluOpType.mult
    ADD = mybir.AluOpType.add
    POW = mybir.AluOpType.pow

    with tile.TileContext(nc) as tc:
        ctx_pools = []   # list of (cm, entered)

        def pool(name, bufs, space=None):
            kw = dict(name=name, bufs=bufs)
            if space:
                kw["space"] = space
            cm = tc.tile_pool(**kw)
            entered = cm.__enter__()
            ctx_pools.append((cm, entered))
            return entered

        def close_pool(entered):
            for i, (cm, e) in enumerate(ctx_pools):
                if e is entered:
                    cm.__exit__(None, None, None)
                    ctx_pools.pop(i)
                    return
            raise KeyError("pool not found")

        consts = pool("consts", 1)
        resid = pool("resid", 1)
        pp_den = pool("pp_den", 1, space="PSUM")  # den [8,512]
        pp_s = pool("pp_s", 2, space="PSUM")      # scores [128,2,512]
        wpool = pool("wpool", 2)    # small working tiles (rope/squares)
        spool = pool("spool", 1)    # [1,n] rs scalars
        pp_all = pool("pp_all", 2, space="PSUM")  # [128,512] proj/V/swap psum
        pp_sq2 = pool("pp_sq2", 1, space="PSUM")  # sumsq [1,512]
        kpool = pool("kpool", 2)

        _cnt = [0]

        def cload(shape, dt, src, via=nc.sync, into=None):
            _cnt[0] += 1
            t = (into or consts).tile(shape, dt, tag=f"c{_cnt[0]}")
            via.dma_start(t[:], src)
            return t

        qT = resid.tile([128, 8, SQ], bf16)         # rope'd normalized Q
        kra = resid.tile([128, 8, CHT, 128], bf16)  # rope'd normalized K (ping)
        krb = resid.tile([128, 8, CHT, 128], bf16)  # (pong)
        v_sb = resid.tile([128, TT, DIM], fp8)
        nacc = resid.tile([128, 8, SQ], f32)
        dacc = resid.tile([8, SQ], f32)

        den_ps = pp_den.tile([8, SQ], f32)

        def rs_broadcast(ps_sq, n):
            """rs = (sumsq_raw/DIM + eps*WS^2)^-0.5 (DVE pow) then bcast.
            y is kept raw (x WS); dividing by the raw rms normalizes WS away
            up to the folded eps."""
            t = spool.tile([1, n], f32, tag="lnm")
            nc.vector.tensor_scalar(t[:], ps_sq[:], 1.0 / DIM,
                                    EPS * WS * WS, MUL, ADD)
            rs = spool.tile([1, n], bf16, tag="rs")
            nc.vector.tensor_scalar(rs[:], t[:], -0.5, 1.0, POW, MUL)
            rsb = wpool.tile([128, n], bf16, tag="rsb")
            nc.gpsimd.partition_broadcast(rsb[:], rs[:])
            return rsb

        def rope_thunks(ysrc, n, rsb_of, ct_of, st_of, out_of,
                        fold_tables=False):
            """per-o rope emission thunks (thunk 0 computes folded tables)."""
            box = {}

            def ro(o):
                if fold_tables:
                    if o == 0:
                        ct_r = wpool.tile([128, n], bf16, tag="ctr")
                        nc.vector.tensor_mul(ct_r[:], ct_of(0), rsb_of())
                        st_r = wpool.tile([128, n], bf16, tag="str")
                        nc.vector.tensor_mul(st_r[:], st_of(0), rsb_of())
                        box["ct"], box["st"] = ct_r, st_r
                    ykn, ct_o, st_o = ysrc[:, o, :], box["ct"][:], box["st"][:]
                else:
                    ykn = wpool.tile([128, n], bf16, tag="ykn")
                    nc.vector.tensor_mul(ykn[:], ysrc[:, o, :], rsb_of())
                    ct_o, st_o = ct_of(o), st_of(o)
                ys = wpool.tile([128, n], bf16, tag="ys")
                nc.vector.tensor_mul(ys[:], ykn, st_o)
                swp = pp_all.tile([128, n], f32, tag="pp")
                nc.tensor.matmul(swp[:], pt_s[:], ys[:])
                yc = wpool.tile([128, n], bf16, tag="yc")
                nc.vector.tensor_mul(yc[:], ykn, ct_o)
                nc.vector.tensor_add(out_of(o), yc[:], swp[:])

            return [lambda o=o: ro(o) for o in range(8)]

        def weave(*lanes):
            """emit lanes with proportional progress (round-robin)."""
            lanes = [list(ln) for ln in lanes if ln]
            total = sum(len(ln) for ln in lanes)
            idx = [0] * len(lanes)
            for step in range(1, total + 1):
                for li, ln in enumerate(lanes):
                    want = (step * len(ln) + total - 1) // total
                    while idx[li] < min(want, len(ln)):
                        ln[idx[li]]()
                        idx[li] += 1

        def sumsq_unit(ydst, o, ps_sq):
            ysq = wpool.tile([128, 512], bf16, tag="ysq")
            nc.gpsimd.tensor_mul(ysq[:], ydst[:, o, :], ydst[:, o, :])
            nc.tensor.matmul(ps_sq[:], ones_s[:], ysq[:],
                             start=(o == 0), stop=(o == 7))

        def unit_K2(hi, lo, mh, ml, ydst, o, ps_sq):
            """two-term fp8 DoubleRow projection block + copy + sumsq."""
            ps = pp_all.tile([128, 512], f32, tag="pp")
            for pr in range(4):
                nc.tensor.matmul(ps[:], hi[:, 2 * pr:2 * pr + 2, o, :],
                                 mh[:, 2 * pr:2 * pr + 2, :],
                                 start=(pr == 0), stop=False, perf_mode=DR)
            for pr in range(4):
                nc.tensor.matmul(ps[:], lo[:, 2 * pr:2 * pr + 2, o, :],
                                 ml[:, 2 * pr:2 * pr + 2, :],
                                 start=False, stop=(pr == 3), perf_mode=DR)
            nc.scalar.activation(ydst[:, o, :], ps[:], AF.Copy)
            sumsq_unit(ydst, o, ps_sq)

        def unit_Kbf(w_s, src, ydst, o, ps_sq):
            """plain bf16 projection block (Q path)."""
            ps = pp_all.tile([128, 512], f32, tag="pp")
            for i in range(8):
                nc.tensor.matmul(ps[:], w_s[:, i, o, :], src[:, i, :],
                                 start=(i == 0), stop=(i == 7))
            nc.scalar.activation(ydst[:, o, :], ps[:], AF.Copy)
            sumsq_unit(ydst, o, ps_sq)

        def unit_V(mh, ml, gt, t):
            """two-term fp8 DoubleRow V projection for key tile t."""
            for oh in range(2):
                ps = pp_all.tile([128, 512], f32, tag="pp")
                for pr in range(4):
                    nc.tensor.matmul(
                        ps[:], mh[:, 2 * pr:2 * pr + 2, t * 128:(t + 1) * 128],
                        wv_s[:, 2 * pr:2 * pr + 2, oh * 512:(oh + 1) * 512],
                        start=(pr == 0), stop=False, perf_mode=DR)
                for pr in range(4):
                    nc.tensor.matmul(
                        ps[:], ml[:, 2 * pr:2 * pr + 2, t * 128:(t + 1) * 128],
                        wl_s[:, 2 * pr:2 * pr + 2, oh * 512:(oh + 1) * 512],
                        start=False, stop=(pr == 3), perf_mode=DR)
                nc.vector.tensor_copy(
                    v_sb[:, gt, oh * 512:(oh + 1) * 512], ps[:])

        def group_SE(c, kr, h, p2):
            """scores + exp for (head h, tile pair p2) of chunk c."""
            ps_s = pp_s.tile([128, 2, 512], f32, tag="ps_s")
            for tt in range(2):
                nc.tensor.matmul(ps_s[:, tt, :], kr[:, h, p2 * 2 + tt, :],
                                 qT[:, h, :])
            g0 = c * CHT + p2 * 2
            if c == NCH - 1:
                # pads live here: per-tile exp bias
                for tt in range(2):
                    nc.scalar.activation(pt_all[:, h, g0 + tt, :],
                                         ps_s[:, tt, :], AF.Exp, scale=SCALE,
                                         bias=mb_s[:, g0 + tt:g0 + tt + 1])
            else:
                nc.scalar.activation(pt_all[:, h, g0:g0 + 2, :], ps_s[:],
                                     AF.Exp, scale=SCALE,
                                     bias=mb_s[:, g0:g0 + 1])

        def den_mm(c, h, p2):
            gp = c * 2 + p2
            nc.tensor.matmul(den_ps[:], sel_s[:, :, h * 8:h * 8 + 8],
                             pt_all[:, h, gp * 2:gp * 2 + 2, :], perf_mode=DR,
                             start=(c == 0 and p2 == 0 and h == 0),
                             stop=(c == NCH - 1 and p2 == 1 and h == 7))

        def numer(h, pp):
            ps_n = pp.tile([128, SQ], f32, tag="pp")
            for p in range(TT // 2):
                nc.tensor.matmul(ps_n[:],
                                 v_sb[:, 2 * p:2 * p + 2, h * 128:(h + 1) * 128],
                                 pt_all[:, h, 2 * p:2 * p + 2, :],
                                 start=(p == 0), stop=(p == TT // 2 - 1),
                                 perf_mode=DR)
            nc.vector.tensor_copy(nacc[:, h, :], ps_n[:])
            if h % 4 == 3:
                nc.gpsimd.dma_start(
                    cat[(h - 3) * 128:(h + 1) * 128, :].rearrange(
                        "(a p) q -> p a q", p=128),
                    nacc[:, h - 3:h + 1, :])

        # ---- loads: SP queue carries the Q/K critical path ----
        wq_s = None  # placed in qpool below
        qpool = pool("qpool", 1)
        wq_s = cload([128, 8, 8, 128], bf16, wq, into=qpool)
        xt_s = qpool.tile([128, 8, SQ], bf16, tag="xt")
        nc.sync.dma_start(xt_s[:], xt)
        wk_s = cload([128, 8, 8, 128], fp8, wk)
        wl2_s = cload([128, 8, 8, 128], fp8, wk2)
        ctq_s = cload([128, 8, SQ], bf16, ctq, via=nc.gpsimd, into=qpool)
        stq_s = cload([128, 8, SQ], bf16, stq, via=nc.gpsimd, into=qpool)
        wv_s = cload([128, 8, DIM], fp8, wv, via=nc.scalar)
        wl_s = cload([128, 8, DIM], fp8, wv2, via=nc.scalar)
        pt_s = cload([128, 128], bf16, pmat, via=nc.gpsimd)
        ones_s = cload([128, 1], bf16, ones_c, via=nc.gpsimd)
        sel_s = cload([128, 2, 64], fp8, sel, via=nc.gpsimd)
        mb_s = cload([128, TT], f32, mbt, via=nc.gpsimd)
        bo_s = cload([128, 8], f32, bo_t, via=nc.gpsimd)

        # =========== pipelined chunk loop (Q phase = prologue) ===========
        cw = CHT * 128
        st = {}

        def s1_load(c):
            c0 = c * cw
            memt = kpool.tile([128, 8, cw], fp8, tag="memt")
            nc.sync.dma_start(
                memt[:], memT[:, c0:c0 + cw].rearrange("(i p) t -> p i t", p=128))
            meml = kpool.tile([128, 8, cw], fp8, tag="meml")
            nc.sync.dma_start(
                meml[:], memL[:, c0:c0 + cw].rearrange("(i p) t -> p i t", p=128))
            cs_t = kpool.tile([128, 2, cw], bf16, tag="cstk")
            nc.sync.dma_start(cs_t[:], cstk[:, :, c0:c0 + cw])
            kr = kra if c % 2 == 0 else krb
            return dict(memt=memt, meml=meml, ctk=cs_t[:, 0, :],
                        stk=cs_t[:, 1, :], kr=kr)

        def chunk_units(c):
            st[c] = s1_load(c)
            ps_sq = pp_sq2.tile([1, cw], f32, tag="pssq")
            st[c]["ps_sq"] = ps_sq
            ykt = kpool.tile([128, 8, 512], bf16, tag="yk")
            st[c]["yk"] = ykt
            units = []
            for o in range(8):
                units.append(lambda o=o, c=c: unit_K2(
                    wk_s, wl2_s, st[c]["memt"], st[c]["meml"],
                    st[c]["yk"], o, st[c]["ps_sq"]))
                if o % 2 == 1:
                    units.append(lambda o=o, c=c: unit_V(
                        st[c]["memt"], st[c]["meml"],
                        c * CHT + o // 2, o // 2))
            return units

        def chunk_rope_thunks(c):
            kr = st[c]["kr"]
            return rope_thunks(
                st[c]["yk"], cw, lambda c=c: st[c]["rsb"][:],
                lambda o, c=c: st[c]["ctk"][:],
                lambda o, c=c: st[c]["stk"][:],
                lambda o, kr=kr: kr[:, o, :, :], fold_tables=True)

        # Q prologue: Q proj, then Q rope woven with chunk-0 proj
        ps_sqq = pp_sq2.tile([1, SQ], f32, tag="pssq")
        yq = kpool.tile([128, 8, SQ], bf16, tag="yk")
        units0 = chunk_units(0)
        for o in range(8):
            unit_Kbf(wq_s, xt_s, yq, o, ps_sqq)
        rsb_q = rs_broadcast(ps_sqq, SQ)
        qrope = rope_thunks(yq, SQ, lambda: rsb_q[:],
                            lambda o: ctq_s[:, o, :], lambda o: stq_s[:, o, :],
                            lambda o: qT[:, o, :])
        weave(units0, qrope)
        close_pool(qpool)
        ptpool = pool("ptpool", 1)
        pt_all = ptpool.tile([128, 8, TT, SQ], fp8)  # exp(scores+shift)
        st[0]["rsb"] = rs_broadcast(st[0]["ps_sq"], cw)

        # pipelined chunk stream: rope(c,o) -> scores/exp(c,h=o-1) -> dens
        # (lagged) with next chunk's proj units spread throughout
        for c in range(NCH):
            last = c == NCH - 1
            ropes = chunk_rope_thunks(c)
            units = chunk_units(c + 1) if not last else []
            kr = kra if c % 2 == 0 else krb
            denq = []          # lagged den emission queue
            ui = 0

            def unit_step(frac, n_slots=12):
                nonlocal ui
                want = min(len(units), (frac * len(units)) // n_slots + 1)
                while ui < want:
                    units[ui]()
                    ui += 1

            slot = 0
            for o in range(8):
                unit_step(slot)
                ropes[o]()
                slot += 1
                for h in ([o - 1] if o >= 1 else []):
                    for p2 in range(2):
                        group_SE(c, kr, h, p2)
                        denq.append((c, h, p2))
                        while len(denq) > 2:
                            den_mm(*denq.pop(0))
                    if last:
                        numer(h, pp_all)
                    unit_step(slot)
                    slot += 1
            for h in (7,):
                for p2 in range(2):
                    group_SE(c, kr, h, p2)
                    denq.append((c, h, p2))
                    while len(denq) > 2:
                        den_mm(*denq.pop(0))
                if last:
                    numer(h, pp_all)
                unit_step(slot)
                slot += 1
            while ui < len(units):
                units[ui]()
                ui += 1
            while denq:
                den_mm(*denq.pop(0))
            if not last:
                st[c + 1]["rsb"] = rs_broadcast(st[c + 1]["ps_sq"], cw)

        nc.scalar.activation(dacc[:], den_ps[:], AF.Copy)
        nc.gpsimd.dma_start(cat[DIM:DIM + HEADS, :], dacc[:])

        # =========== reduce across cores ===========
        if _sim:
            nc.gpsimd.dma_start(cat_sh[0:512, :], cat[0:512, :])
            nc.gpsimd.dma_start(cat_sh[512:DIM + HEADS, :],
                                cat[512:DIM + HEADS, :])
        else:
            nc.gpsimd.collective_compute(
                "AllReduce", mybir.AluOpType.add,
                replica_groups=[list(range(N_CORES))],
                ins=[cat[:]], outs=[cat_sh[:]])

        for p in (kpool, pp_sq2, pp_all, spool, wpool, pp_s):
            close_pool(p)

        # =========== per-core output projection on its query slice ==========
        tail = pool("tail", 1)
        pp_t = pool("pp_t", 2, space="PSUM")
        wo_s = cload([128, 8, 8, 128], bf16, wo, into=tail)
        nred = tail.tile([128, 8, QS], f32)
        dred = tail.tile([1, HEADS, QS], f32)
        pid = nc.sync.partition_id()
        qoff = pid * QS
        nc.sync.dma_start(
            nred[:],
            cat_sh[0:DIM, bass.ds(qoff, QS)].rearrange("(h p) q -> p h q", p=128))
        nc.sync.dma_start(dred[:], cat_sh[DIM:DIM + HEADS, bass.ds(qoff, QS)])
        rd = tail.tile([1, HEADS, QS], f32)
        nc.vector.reciprocal(rd[:], dred[:])
        rdb = tail.tile([128, HEADS, QS], f32)
        nc.gpsimd.partition_broadcast(rdb[:], rd[:])
        nsc = tail.tile([128, 8, QS], bf16)
        nc.vector.tensor_mul(nsc[:], nred[:], rdb[:])
        out_sb = tail.tile([128, 8, QS], f32)
        for e in range(8):
            ps_o = pp_t.tile([128, QS], f32, tag="ppo")
            for o in range(8):
                nc.tensor.matmul(ps_o[:], wo_s[:, o, e, :], nsc[:, o, :],
                                 start=(o == 0), stop=(o == 7))
            nc.scalar.activation(out_sb[:, e, :], ps_o[:], AF.Identity,
                                 scale=1.0 / (WS * WS), bias=bo_s[:, e:e + 1])
        nc.sync.dma_start(
            outT.rearrange("(e p) q -> p e q", p=128)[:, :, 0:QS], out_sb[:])

        for cm, _ in reversed(ctx_pools):
            cm.__exit__(None, None, None)

    nc.compile()
    _cache[key] = nc
    _cache["nc"] = nc
    return nc


def _skc_for(nkeep):
    return max(CHT * 128, int(np.ceil(nkeep / (N_CORES * 512))) * 512)


def _prep(x, mem, mask, cos_q, sin_q, cos_k, sin_k,
          Wq, bq, Wk, bk, Wv, bv, Wo, bo, gq, gk):
    import ml_dtypes
    f = np.float32
    bf = ml_dtypes.bfloat16
    f8 = ml_dtypes.float8_e4m3
    x = np.asarray(x, f).reshape(SQ, DIM)
    mem = np.asarray(mem, f).reshape(-1, DIM)
    mask = np.asarray(mask).reshape(-1)
    cos_q = np.asarray(cos_q, f)
    sin_q = np.asarray(sin_q, f)
    cos_k = np.asarray(cos_k, f)
    sin_k = np.asarray(sin_k, f)
    Wq, Wk, Wv, Wo = (np.asarray(w, f) for w in (Wq, Wk, Wv, Wo))
    bq, bk, bv, bo, gq, gk = (np.asarray(v, f) for v in (bq, bk, bv, bo, gq, gk))

    if not np.allclose(gk, 1.0):
        gkp = gk.reshape(-1, 2)
        assert np.allclose(gkp[:, 0], gkp[:, 1]), "unsupported non-pairwise gk"
    assert np.allclose(bq, 0) and np.allclose(bk, 0), \
        "kernel specialized for zero q/k biases"

    idx = np.flatnonzero(mask)
    nkeep = len(idx)
    skc = _skc_for(nkeep)
    TT = skc // 128
    base, rem = divmod(nkeep, N_CORES)
    counts = [base + (1 if c < rem else 0) for c in range(N_CORES)]
    offs = np.concatenate([[0], np.cumsum(counts)])

    def tile_w(WT, dt):  # [1024,1024] (in,out of W.T) -> [p, i, o, m], scaled
        return np.ascontiguousarray(
            WT.reshape(8, 128, 8, 128).transpose(1, 0, 2, 3)).astype(dt)

    def two_term(WT):
        hi = (WT).astype(f8)
        lo = ((WT - hi.astype(f)) * 16.0).astype(f8)
        return hi.astype(f), lo.astype(f)

    ii = np.arange(128)
    jj = ii // 2
    partner = ii ^ 1

    # fold gq (and pairwise gk) into the q rope tables; sin pairs with
    # partner's gq
    gq_t = (gq * gk).reshape(8, 128)
    gq_sin = (gq.reshape(8, 128)[:, partner] * gk.reshape(8, 128))
    cq = cos_q[:, jj].T                # [128, SQ]
    sq = sin_q[:, jj].T
    ctq = np.ascontiguousarray(
        (cq[None, :, :] * gq_t[:, :, None]).transpose(1, 0, 2)).astype(bf)
    stq = np.ascontiguousarray(
        (sq[None, :, :] * gq_sin[:, :, None]).transpose(1, 0, 2)).astype(bf)

    PT = np.zeros((128, 128), f)
    even = ii[ii % 2 == 0]
    PT[even + 1, even] = -1.0
    PT[even, even + 1] = 1.0

    selm = np.zeros((128, 2, 64), f)
    for h in range(8):
        selm[:, :, h * 8 + h] = 1.0

    bo_f = bo + Wo @ bv

    wkh, wkl = two_term(Wk.T * WS)
    wvh, wvl = two_term(Wv.T * WS)

    shared = {
        "xt": np.ascontiguousarray(
            x.T.reshape(8, 128, SQ).transpose(1, 0, 2)).astype(bf),
        "wq": tile_w(Wq.T * WS, bf),
        "wk": tile_w(wkh, f8), "wk2": tile_w(wkl, f8),
        "wo": tile_w(Wo.T * WS, bf),
        "wv": np.ascontiguousarray(
            wvh.reshape(8, 128, DIM).transpose(1, 0, 2)).astype(f8),
        "wv2": np.ascontiguousarray(
            wvl.reshape(8, 128, DIM).transpose(1, 0, 2)).astype(f8),
        "ctq": ctq, "stq": stq,
        "bo_t": np.ascontiguousarray(bo_f.reshape(8, 128).T),
        "pmat": PT.astype(bf),
        "ones_c": np.ones((128, 1), bf),
        "sel": selm.astype(f8),
    }

    ckT = cos_k[:, jj].T.astype(f)     # [128, SK]
    skT = sin_k[:, jj].T.astype(f)

    in_maps = []
    for c in range(N_CORES):
        keys = idx[offs[c]:offs[c + 1]]
        n = len(keys)
        m = dict(shared)
        memc = np.zeros((DIM, skc), f8)
        memc[:, :n] = mem[keys].T.astype(f8)
        meml = np.zeros((DIM, skc), f8)
        meml[:, :n] = (mem[keys].T / 16.0).astype(f8)
        cstk = np.zeros((HD, 2, skc), bf)
        cstk[:, 0, :n] = ckT[:, keys].astype(bf)
        cstk[:, 1, :n] = skT[:, keys].astype(bf)
        mb = np.full((skc,), PADB, f)
        mb[:n] = SHIFT
        m["memT"] = memc
        m["memL"] = meml
        m["cstk"] = cstk
        m["mbt"] = np.ascontiguousarray(mb.reshape(TT, 128).T)
        in_maps.append(m)
    return in_maps


def _assemble(outTs):
    parts = [np.asarray(outTs[c])[:, 0:QS].T for c in range(N_CORES)]
    out = np.concatenate(parts, axis=0)
    return out[None].astype(np.float32)


def kernel(**inputs):
    from concourse.bass_utils import run_bass_kernel_spmd
    in_maps = _prep(**inputs)
    skc = in_maps[0]["memT"].shape[1]
    nc = _build(skc)
    res = run_bass_kernel_spmd(nc, in_maps, list(range(N_CORES)))
    return _assemble([res.results[c]["outT"] for c in range(N_CORES)])


# revision 44
# speedup vs baseline: 1.0662x; 1.0241x over previous
"""Trainium2 Bass kernel for nn_MemoryRetriever (cross-attention memory retriever).

Strategy (v3):
- Host-side mask compaction: masked-out keys (~50%) are dropped on the host;
  survivors are dealt evenly to the 8 cores (zero-padded to SKC keys/core,
  pads confined to each core's last 512-key chunk and killed by a -30 exp
  bias).
- Precision plan (rel-err budget ~1.6e-2): everything is bf16 except three
  fp8 uses that attention averaging washes out: the mem operand of the K/V
  projections, the V projection weights (two-term hi+lo fp8, lo applied to
  mem/16 so both DoubleRow matmuls accumulate in one PSUM group), and the
  exp outputs pt (which feed fp8 DoubleRow numerator and denominator
  reductions). K projection weights use the same two-term fp8 split.
  Q path, scores, output path are bf16; the collective payload is bf16.
- PE: K/V projections run at 2x bf16 speed via paired-fp8 DoubleRow; the
  denominator is a DoubleRow matmul against a head-selector so no vector
  accumulation is needed; numerator is DoubleRow over key-tile pairs.
- Schedule: per-chunk software pipeline -- rope(o) unlocks head h=o-1
  scores+exp immediately; next chunk's projection units and lagged
  denominator matmuls fill all gaps. GPSIMD only touches SBUF (squares,
  broadcasts); PSUM reads happen on PE/Act/DVE only.
"""

import os
import sys
import numpy as np

sys.path.insert(0, "/opt/trn_rl_repo")

DIM = 1024
HEADS = 8
HD = 128
SQ = 512
N_CORES = 8
QS = SQ // N_CORES
EPS = 1e-6
SCALE = 1.0 / np.sqrt(128.0)
WS = 16.0            # host-side weight scale (fp8 subnormal avoidance)
SHIFT = -3.0         # exp(score + SHIFT): keeps fp8e4 pt in range
PADB = -30.0         # exp bias for padded keys
CHT = 4              # key tiles (128) per chunk

_cache = {}


def _build(skc=2048):
    key = ("nc", skc)
    if key in _cache:
        return _cache[key]

    import concourse.bass as bass
    import concourse.tile as tile
    from concourse import mybir, bacc

    f32 = mybir.dt.float32
    bf16 = mybir.dt.bfloat16
    fp8 = mybir.dt.float8e4
    AF = mybir.ActivationFunctionType
    DR = mybir.MatmulPerfMode.DoubleRow

    TT = skc // 128          # key tiles per core
    NCH = TT // CHT          # chunks per core
    assert skc % (CHT * 128) == 0

    _sim = os.environ.get("KSIM", "0") == "1"

    nc = bacc.Bacc("TRN2", target_bir_lowering=False, debug=False,
                   num_devices=N_CORES)

    def din(name, shape, dt=f32):
        return nc.dram_tensor(name, list(shape), dt, kind="ExternalInput").ap()

    # per-core sharded inputs
    memT = din("memT", [DIM, skc], fp8)       # compacted mem shard
    memL = din("memL", [DIM, skc], fp8)       # mem/16 (two-term lo operand)
    cstk = din("cstk", [HD, 2, skc], bf16)    # K rope cos/sin (pair-dup rows)
    mbt = din("mbt", [128, TT])               # exp bias per (key%128, tile)
    # shared inputs
    xt = din("xt", [128, 8, SQ], bf16)        # x.T tiled [p,i,q]
    wq = din("wq", [128, 8, 8, 128], bf16)    # [p,i,o,m] = WS*Wq.T[i*128+p, .]
    wk = din("wk", [128, 8, 8, 128], fp8)     # two-term hi
    wk2 = din("wk2", [128, 8, 8, 128], fp8)   # two-term lo (16x residual)
    wv = din("wv", [128, 8, DIM], fp8)        # [p,i,d] hi
    wv2 = din("wv2", [128, 8, DIM], fp8)      # lo
    wo = din("wo", [128, 8, 8, 128], bf16)
    ctq = din("ctq", [128, 8, SQ], bf16)      # q rope cos (gq*gk folded)
    stq = din("stq", [128, 8, SQ], bf16)
    bo_t = din("bo_t", [128, 8])              # bo + Wo@bv folded
    pmat = din("pmat", [128, 128], bf16)      # P.T for rope pair swap (+-1)
    ones_c = din("ones_c", [128, 1], bf16)
    sel = din("sel", [128, 2, 64], fp8)       # den head selector

    outT = nc.dram_tensor("outT", [DIM, SQ], f32, kind="ExternalOutput").ap()
    cat = nc.dram_tensor("cat", [DIM + HEADS, SQ], f32)
    cat_sh = nc.dram_tensor("cat_sh", [DIM + HEADS, SQ], f32,
                            addr_space="Shared")

    MUL = mybir.AluOpType.mult
    ADD = mybir.AluOpType.add
    POW = mybir.AluOpType.pow

    with tile.TileContext(nc) as tc:
        ctx_pools = []   # list of (cm, entered)

        def pool(name, bufs, space=None):
            kw = dict(name=name, bufs=bufs)
            if space:
                kw["space"] = space
            cm = tc.tile_pool(**kw)
            entered = cm.__enter__()
            ctx_pools.append((cm, entered))
            return entered

        def close_pool(entered):
            for i, (cm, e) in enumerate(ctx_pools):
                if e is entered:
                    cm.__exit__(None, None, None)
                    ctx_pools.pop(i)
                    return
            raise KeyError("pool not found")

        consts = pool("consts", 1)
        resid = pool("resid", 1)
        pp_den = pool("pp_den", 1, space="PSUM")  # den [8,512]
        pp_s = pool("pp_s", 2, space="PSUM")      # scores [128,2,512]
        wpool = pool("wpool", 2)    # small working tiles (rope/squares)
        spool = pool("spool", 1)    # [1,n] rs scalars
        pp_all = pool("pp_all", 2, space="PSUM")  # [128,512] proj/V/swap psum
        pp_sq2 = pool("pp_sq2", 1, space="PSUM")  # sumsq [1,512]
        kpool = pool("kpool", 2)

        _cnt = [0]

        def cload(shape, dt, src, via=nc.sync, into=None):
            _cnt[0] += 1
            t = (into or consts).tile(shape, dt, tag=f"c{_cnt[0]}")
            via.dma_start(t[:], src)
            return t

        qT = resid.tile([128, 8, SQ], bf16)         # rope'd normalized Q
        kra = resid.tile([128, 8, CHT, 128], bf16)  # rope'd normalized K (ping)
        krb = resid.tile([128, 8, CHT, 128], bf16)  # (pong)
        v_sb = resid.tile([128, TT, DIM], fp8)
        nacc = resid.tile([128, 8, SQ], f32)
        dacc = resid.tile([8, SQ], f32)

        den_ps = pp_den.tile([8, SQ], f32)

        def rs_broadcast(ps_sq, n):
            """rs = (sumsq_raw/DIM + eps*WS^2)^-0.5 (DVE pow) then bcast.
            y is kept raw (x WS); dividing by the raw rms normalizes WS away
            up to the folded eps."""
            t = spool.tile([1, n], f32, tag="lnm")
            nc.vector.tensor_scalar(t[:], ps_sq[:], 1.0 / DIM,
                                    EPS * WS * WS, MUL, ADD)
            rs = spool.tile([1, n], bf16, tag="rs")
            nc.vector.tensor_scalar(rs[:], t[:], -0.5, 1.0, POW, MUL)
            rsb = wpool.tile([128, n], bf16, tag="rsb")
            nc.gpsimd.partition_broadcast(rsb[:], rs[:])
            return rsb

        def rope_thunks(ysrc, n, rsb_of, ct_of, st_of, out_of,
                        fold_tables=False):
            """per-o rope emission thunks (thunk 0 computes folded tables)."""
            box = {}

            def ro(o):
                if fold_tables:
                    if o == 0:
                        ct_r = wpool.tile([128, n], bf16, tag="ctr")
                        nc.vector.tensor_mul(ct_r[:], ct_of(0), rsb_of())
                        st_r = wpool.tile([128, n], bf16, tag="str")
                        nc.vector.tensor_mul(st_r[:], st_of(0), rsb_of())
                        box["ct"], box["st"] = ct_r, st_r
                    ykn, ct_o, st_o = ysrc[:, o, :], box["ct"][:], box["st"][:]
                else:
                    ykn = wpool.tile([128, n], bf16, tag="ykn")
                    nc.vector.tensor_mul(ykn[:], ysrc[:, o, :], rsb_of())
                    ct_o, st_o = ct_of(o), st_of(o)
                ys = wpool.tile([128, n], bf16, tag="ys")
                nc.vector.tensor_mul(ys[:], ykn, st_o)
                swp = pp_all.tile([128, n], f32, tag="pp")
                nc.tensor.matmul(swp[:], pt_s[:], ys[:])
                yc = wpool.tile([128, n], bf16, tag="yc")
                nc.vector.tensor_mul(yc[:], ykn, ct_o)
                nc.vector.tensor_add(out_of(o), yc[:], swp[:])

            return [lambda o=o: ro(o) for o in range(8)]

        def weave(*lanes):
            """emit lanes with proportional progress (round-robin)."""
            lanes = [list(ln) for ln in lanes if ln]
            total = sum(len(ln) for ln in lanes)
            idx = [0] * len(lanes)
            for step in range(1, total + 1):
                for li, ln in enumerate(lanes):
                    want = (step * len(ln) + total - 1) // total
                    while idx[li] < min(want, len(ln)):
                        ln[idx[li]]()
                        idx[li] += 1

        def sumsq_unit(ydst, o, ps_sq):
            ysq = wpool.tile([128, 512], bf16, tag="ysq")
            nc.gpsimd.tensor_mul(ysq[:], ydst[:, o, :], ydst[:, o, :])
            nc.tensor.matmul(ps_sq[:], ones_s[:], ysq[:],
                             start=(o == 0), stop=(o == 7))

        def unit_K2(hi, lo, mh, ml, ydst, o, ps_sq):
            """two-term fp8 DoubleRow projection block + copy + sumsq."""
            ps = pp_all.tile([128, 512], f32, tag="pp")
            for pr in range(4):
                nc.tensor.matmul(ps[:], hi[:, 2 * pr:2 * pr + 2, o, :],
                                 mh[:, 2 * pr:2 * pr + 2, :],
                                 start=(pr == 0), stop=False, perf_mode=DR)
            for pr in range(4):
                nc.tensor.matmul(ps[:], lo[:, 2 * pr:2 * pr + 2, o, :],
                                 ml[:, 2 * pr:2 * pr + 2, :],
                                 start=False, stop=(pr == 3), perf_mode=DR)
            nc.scalar.activation(ydst[:, o, :], ps[:], AF.Copy)
            sumsq_unit(ydst, o, ps_sq)

        def unit_Kbf(w_s, src, ydst, o, ps_sq):
            """plain bf16 projection block (Q path)."""
            ps = pp_all.tile([128, 512], f32, tag="pp")
            for i in range(8):
                nc.tensor.matmul(ps[:], w_s[:, i, o, :], src[:, i, :],
                                 start=(i == 0), stop=(i == 7))
            nc.scalar.activation(ydst[:, o, :], ps[:], AF.Copy)
            sumsq_unit(ydst, o, ps_sq)

        def unit_V(mh, ml, gt, t):
            """two-term fp8 DoubleRow V projection for key tile t."""
            for oh in range(2):
                ps = pp_all.tile([128, 512], f32, tag="pp")
                for pr in range(4):
                    nc.tensor.matmul(
                        ps[:], mh[:, 2 * pr:2 * pr + 2, t * 128:(t + 1) * 128],
                        wv_s[:, 2 * pr:2 * pr + 2, oh * 512:(oh + 1) * 512],
                        start=(pr == 0), stop=False, perf_mode=DR)
                for pr in range(4):
                    nc.tensor.matmul(
                        ps[:], ml[:, 2 * pr:2 * pr + 2, t * 128:(t + 1) * 128],
                        wl_s[:, 2 * pr:2 * pr + 2, oh * 512:(oh + 1) * 512],
                        start=False, stop=(pr == 3), perf_mode=DR)
                nc.vector.tensor_copy(
                    v_sb[:, gt, oh * 512:(oh + 1) * 512], ps[:])

        def group_SE(c, kr, h, p2):
            """scores + exp for (head h, tile pair p2) of chunk c."""
            ps_s = pp_s.tile([128, 2, 512], f32, tag="ps_s")
            for tt in range(2):
                nc.tensor.matmul(ps_s[:, tt, :], kr[:, h, p2 * 2 + tt, :],
                                 qT[:, h, :])
            g0 = c * CHT + p2 * 2
            if c == NCH - 1:
                # pads live here: per-tile exp bias
                for tt in range(2):
                    nc.scalar.activation(pt_all[:, h, g0 + tt, :],
                                         ps_s[:, tt, :], AF.Exp, scale=SCALE,
                                         bias=mb_s[:, g0 + tt:g0 + tt + 1])
            else:
                nc.scalar.activation(pt_all[:, h, g0:g0 + 2, :], ps_s[:],
                                     AF.Exp, scale=SCALE,
                                     bias=mb_s[:, g0:g0 + 1])

        def den_mm(c, h, p2):
            gp = c * 2 + p2
            nc.tensor.matmul(den_ps[:], sel_s[:, :, h * 8:h * 8 + 8],
                             pt_all[:, h, gp * 2:gp * 2 + 2, :], perf_mode=DR,
                             start=(c == 0 and p2 == 0 and h == 0),
                             stop=(c == NCH - 1 and p2 == 1 and h == 7))

        def numer(h, pp):
            ps_n = pp.tile([128, SQ], f32, tag="pp")
            for p in range(TT // 2):
                nc.tensor.matmul(ps_n[:],
                                 v_sb[:, 2 * p:2 * p + 2, h * 128:(h + 1) * 128],
                                 pt_all[:, h, 2 * p:2 * p + 2, :],
                                 start=(p == 0), stop=(p == TT // 2 - 1),
                                 perf_mode=DR)
            nc.vector.tensor_copy(nacc[:, h, :], ps_n[:])
            if h % 4 == 3:
                nc.gpsimd.dma_start(
                    cat[(h - 3) * 128:(h + 1) * 128, :].rearrange(
                        "(a p) q -> p a q", p=128),
                    nacc[:, h - 3:h + 1, :])
                if h == 3 and _sim:
                    nc.gpsimd.dma_start(cat_sh[0:512, :], cat[0:512, :])

        # ---- loads: SP queue carries the Q/K critical path ----
        wq_s = None  # placed in qpool below
        qpool = pool("qpool", 1)
        _cnt[0] += 1
        wq_s = qpool.tile([128, 8, 8, 128], bf16, tag=f"c{_cnt[0]}")
        nc.sync.dma_start(wq_s[:, :, 0:4, :], wq[:, :, 0:4, :])
        nc.sync.dma_start(wq_s[:, :, 4:8, :], wq[:, :, 4:8, :])
        xt_s = qpool.tile([128, 8, SQ], bf16, tag="xt")
        nc.scalar.dma_start(xt_s[:], xt)
        wk_s = cload([128, 8, 8, 128], fp8, wk)
        wl2_s = cload([128, 8, 8, 128], fp8, wk2)
        ctq_s = cload([128, 8, SQ], bf16, ctq, via=nc.gpsimd, into=qpool)
        stq_s = cload([128, 8, SQ], bf16, stq, via=nc.gpsimd, into=qpool)
        wv_s = cload([128, 8, DIM], fp8, wv, via=nc.scalar)
        wl_s = cload([128, 8, DIM], fp8, wv2, via=nc.scalar)
        pt_s = cload([128, 128], bf16, pmat, via=nc.gpsimd)
        ones_s = cload([128, 1], bf16, ones_c, via=nc.gpsimd)
        sel_s = cload([128, 2, 64], fp8, sel, via=nc.gpsimd)
        mb_s = cload([128, TT], f32, mbt, via=nc.gpsimd)
        bo_s = cload([128, 8], f32, bo_t, via=nc.gpsimd)

        # =========== pipelined chunk loop (Q phase = prologue) ===========
        cw = CHT * 128
        st = {}

        def s1_load(c):
            c0 = c * cw
            memt = kpool.tile([128, 8, cw], fp8, tag="memt")
            nc.sync.dma_start(
                memt[:], memT[:, c0:c0 + cw].rearrange("(i p) t -> p i t", p=128))
            meml = kpool.tile([128, 8, cw], fp8, tag="meml")
            nc.sync.dma_start(
                meml[:], memL[:, c0:c0 + cw].rearrange("(i p) t -> p i t", p=128))
            cs_t = kpool.tile([128, 2, cw], bf16, tag="cstk")
            nc.sync.dma_start(cs_t[:], cstk[:, :, c0:c0 + cw])
            kr = kra if c % 2 == 0 else krb
            return dict(memt=memt, meml=meml, ctk=cs_t[:, 0, :],
                        stk=cs_t[:, 1, :], kr=kr)

        def chunk_units(c):
            st[c] = s1_load(c)
            ps_sq = pp_sq2.tile([1, cw], f32, tag="pssq")
            st[c]["ps_sq"] = ps_sq
            ykt = kpool.tile([128, 8, 512], bf16, tag="yk")
            st[c]["yk"] = ykt
            units = []
            for o in range(8):
                units.append(lambda o=o, c=c: unit_K2(
                    wk_s, wl2_s, st[c]["memt"], st[c]["meml"],
                    st[c]["yk"], o, st[c]["ps_sq"]))
                if o % 2 == 1:
                    units.append(lambda o=o, c=c: unit_V(
                        st[c]["memt"], st[c]["meml"],
                        c * CHT + o // 2, o // 2))
            return units

        def chunk_rope_thunks(c):
            kr = st[c]["kr"]
            return rope_thunks(
                st[c]["yk"], cw, lambda c=c: st[c]["rsb"][:],
                lambda o, c=c: st[c]["ctk"][:],
                lambda o, c=c: st[c]["stk"][:],
                lambda o, kr=kr: kr[:, o, :, :], fold_tables=True)

        # Q prologue: Q proj, then Q rope woven with chunk-0 proj
        ps_sqq = pp_sq2.tile([1, SQ], f32, tag="pssq")
        yq = kpool.tile([128, 8, SQ], bf16, tag="yk")
        units0 = chunk_units(0)
        for o in range(8):
            unit_Kbf(wq_s, xt_s, yq, o, ps_sqq)
        rsb_q = rs_broadcast(ps_sqq, SQ)
        qrope = rope_thunks(yq, SQ, lambda: rsb_q[:],
                            lambda o: ctq_s[:, o, :], lambda o: stq_s[:, o, :],
                            lambda o: qT[:, o, :])
        weave(units0, qrope)
        close_pool(qpool)
        ptpool = pool("ptpool", 1)
        pt_all = ptpool.tile([128, 8, TT, SQ], fp8)  # exp(scores+shift)
        st[0]["rsb"] = rs_broadcast(st[0]["ps_sq"], cw)

        # pipelined chunk stream: rope(c,o) -> scores/exp(c,h=o-1) -> dens
        # (lagged) with next chunk's proj units spread throughout
        for c in range(NCH):
            last = c == NCH - 1
            ropes = chunk_rope_thunks(c)
            units = chunk_units(c + 1) if not last else []
            kr = kra if c % 2 == 0 else krb
            denq = []          # lagged den emission queue
            ui = 0

            def unit_step(frac, n_slots=12):
                nonlocal ui
                want = min(len(units), (frac * len(units)) // n_slots + 1)
                while ui < want:
                    units[ui]()
                    ui += 1

            slot = 0
            for o in range(8):
                unit_step(slot)
                ropes[o]()
                slot += 1
                for h in ([o - 1] if o >= 1 else []):
                    for p2 in range(2):
                        group_SE(c, kr, h, p2)
                        denq.append((c, h, p2))
                        while len(denq) > 2:
                            den_mm(*denq.pop(0))
                    if last:
                        numer(h, pp_all)
                    unit_step(slot)
                    slot += 1
            for h in (7,):
                for p2 in range(2):
                    group_SE(c, kr, h, p2)
                    denq.append((c, h, p2))
                    while len(denq) > 2:
                        den_mm(*denq.pop(0))
                if last:
                    numer(h, pp_all)
                unit_step(slot)
                slot += 1
            while ui < len(units):
                units[ui]()
                ui += 1
            while denq:
                den_mm(*denq.pop(0))
            if not last:
                st[c + 1]["rsb"] = rs_broadcast(st[c + 1]["ps_sq"], cw)

        nc.scalar.activation(dacc[:], den_ps[:], AF.Copy)
        nc.gpsimd.dma_start(cat[DIM:DIM + HEADS, :], dacc[:])

        # =========== reduce across cores ===========
        if _sim:
            nc.gpsimd.dma_start(cat_sh[512:DIM + HEADS, :],
                                cat[512:DIM + HEADS, :])
        else:
            nc.gpsimd.collective_compute(
                "AllReduce", mybir.AluOpType.add,
                replica_groups=[list(range(N_CORES))],
                ins=[cat[:]], outs=[cat_sh[:]])

        for p in (kpool, pp_sq2, pp_all, spool, wpool, pp_s):
            close_pool(p)

        # =========== per-core output projection on its query slice ==========
        tail = pool("tail", 1)
        pp_t = pool("pp_t", 2, space="PSUM")
        wo_s = cload([128, 8, 8, 128], bf16, wo, into=tail)
        nred = tail.tile([128, 8, QS], f32)
        dred = tail.tile([1, HEADS, QS], f32)
        pid = nc.sync.partition_id()
        qoff = pid * QS
        nc.sync.dma_start(
            nred[:],
            cat_sh[0:DIM, bass.ds(qoff, QS)].rearrange("(h p) q -> p h q", p=128))
        nc.sync.dma_start(dred[:], cat_sh[DIM:DIM + HEADS, bass.ds(qoff, QS)])
        rd = tail.tile([1, HEADS, QS], f32)
        nc.vector.reciprocal(rd[:], dred[:])
        rdb = tail.tile([128, HEADS, QS], f32)
        nc.gpsimd.partition_broadcast(rdb[:], rd[:])
        nsc = tail.tile([128, 8, QS], bf16)
        nc.vector.tensor_mul(nsc[:], nred[:], rdb[:])
        out_sb = tail.tile([128, 8, QS], f32)
        for e in range(8):
            ps_o = pp_t.tile([128, QS], f32, tag="ppo")
            for o in range(8):
                nc.tensor.matmul(ps_o[:], wo_s[:, o, e, :], nsc[:, o, :],
                                 start=(o == 0), stop=(o == 7))
            nc.scalar.activation(out_sb[:, e, :], ps_o[:], AF.Identity,
                                 scale=1.0 / (WS * WS), bias=bo_s[:, e:e + 1])
        nc.sync.dma_start(
            outT.rearrange("(e p) q -> p e q", p=128)[:, :, 0:QS], out_sb[:])

        for cm, _ in reversed(ctx_pools):
            cm.__exit__(None, None, None)

    nc.compile()
    _cache[key] = nc
    _cache["nc"] = nc
    return nc


def _skc_for(nkeep):
    return max(CHT * 128, int(np.ceil(nkeep / (N_CORES * 512))) * 512)


def _prep(x, mem, mask, cos_q, sin_q, cos_k, sin_k,
          Wq, bq, Wk, bk, Wv, bv, Wo, bo, gq, gk):
    import ml_dtypes
    f = np.float32
    bf = ml_dtypes.bfloat16
    f8 = ml_dtypes.float8_e4m3
    x = np.asarray(x, f).reshape(SQ, DIM)
    mem = np.asarray(mem, f).reshape(-1, DIM)
    mask = np.asarray(mask).reshape(-1)
    cos_q = np.asarray(cos_q, f)
    sin_q = np.asarray(sin_q, f)
    cos_k = np.asarray(cos_k, f)
    sin_k = np.asarray(sin_k, f)
    Wq, Wk, Wv, Wo = (np.asarray(w, f) for w in (Wq, Wk, Wv, Wo))
    bq, bk, bv, bo, gq, gk = (np.asarray(v, f) for v in (bq, bk, bv, bo, gq, gk))

    if not np.allclose(gk, 1.0):
        gkp = gk.reshape(-1, 2)
        assert np.allclose(gkp[:, 0], gkp[:, 1]), "unsupported non-pairwise gk"
    assert np.allclose(bq, 0) and np.allclose(bk, 0), \
        "kernel specialized for zero q/k biases"

    idx = np.flatnonzero(mask)
    nkeep = len(idx)
    skc = _skc_for(nkeep)
    TT = skc // 128
    base, rem = divmod(nkeep, N_CORES)
    counts = [base + (1 if c < rem else 0) for c in range(N_CORES)]
    offs = np.concatenate([[0], np.cumsum(counts)])

    def tile_w(WT, dt):  # [1024,1024] (in,out of W.T) -> [p, i, o, m], scaled
        return np.ascontiguousarray(
            WT.reshape(8, 128, 8, 128).transpose(1, 0, 2, 3)).astype(dt)

    def two_term(WT):
        hi = (WT).astype(f8)
        lo = ((WT - hi.astype(f)) * 16.0).astype(f8)
        return hi.astype(f), lo.astype(f)

    ii = np.arange(128)
    jj = ii // 2
    partner = ii ^ 1

    # fold gq (and pairwise gk) into the q rope tables; sin pairs with
    # partner's gq
    gq_t = (gq * gk).reshape(8, 128)
    gq_sin = (gq.reshape(8, 128)[:, partner] * gk.reshape(8, 128))
    cq = cos_q[:, jj].T                # [128, SQ]
    sq = sin_q[:, jj].T
    ctq = np.ascontiguousarray(
        (cq[None, :, :] * gq_t[:, :, None]).transpose(1, 0, 2)).astype(bf)
    stq = np.ascontiguousarray(
        (sq[None, :, :] * gq_sin[:, :, None]).transpose(1, 0, 2)).astype(bf)

    PT = np.zeros((128, 128), f)
    even = ii[ii % 2 == 0]
    PT[even + 1, even] = -1.0
    PT[even, even + 1] = 1.0

    selm = np.zeros((128, 2, 64), f)
    for h in range(8):
        selm[:, :, h * 8 + h] = 1.0

    bo_f = bo + Wo @ bv

    wkh, wkl = two_term(Wk.T * WS)
    wvh, wvl = two_term(Wv.T * WS)

    shared = {
        "xt": np.ascontiguousarray(
            x.T.reshape(8, 128, SQ).transpose(1, 0, 2)).astype(bf),
        "wq": tile_w(Wq.T * WS, bf),
        "wk": tile_w(wkh, f8), "wk2": tile_w(wkl, f8),
        "wo": tile_w(Wo.T * WS, bf),
        "wv": np.ascontiguousarray(
            wvh.reshape(8, 128, DIM).transpose(1, 0, 2)).astype(f8),
        "wv2": np.ascontiguousarray(
            wvl.reshape(8, 128, DIM).transpose(1, 0, 2)).astype(f8),
        "ctq": ctq, "stq": stq,
        "bo_t": np.ascontiguousarray(bo_f.reshape(8, 128).T),
        "pmat": PT.astype(bf),
        "ones_c": np.ones((128, 1), bf),
        "sel": selm.astype(f8),
    }

    ckT = cos_k[:, jj].T.astype(f)     # [128, SK]
    skT = sin_k[:, jj].T.astype(f)

    in_maps = []
    for c in range(N_CORES):
        keys = idx[offs[c]:offs[c + 1]]
        n = len(keys)
        m = dict(shared)
        memc = np.zeros((DIM, skc), f8)
        memc[:, :n] = mem[keys].T.astype(f8)
        meml = np.zeros((DIM, skc), f8)
        meml[:, :n] = (mem[keys].T / 16.0).astype(f8)
        cstk = np.zeros((HD, 2, skc), bf)
        cstk[:, 0, :n] = ckT[:, keys].astype(bf)
        cstk[:, 1, :n] = skT[:, keys].astype(bf)
        mb = np.full((skc,), PADB, f)
        mb[:n] = SHIFT
        m["memT"] = memc
        m["memL"] = meml
        m["cstk"] = cstk
        m["mbt"] = np.ascontiguousarray(mb.reshape(TT, 128).T)
        in_maps.append(m)
    return in_maps


def _assemble(outTs):
    parts = [np.asarray(outTs[c])[:, 0:QS].T for c in range(N_CORES)]
    out = np.concatenate(parts, axis=0)
    return out[None].astype(np.float32)


def kernel(**inputs):
    from concourse.bass_utils import run_bass_kernel_spmd
    in_maps = _prep(**inputs)
    skc = in_maps[0]["memT"].shape[1]
    nc = _build(skc)
    res = run_bass_kernel_spmd(nc, in_maps, list(range(N_CORES)))
    return _assemble([res.results[c]["outT"] for c in range(N_CORES)])
